# revision 4
# baseline (speedup 1.0000x reference)
"""EnhancedAdaptiveLoRAPooling fused kernel for 8x Trainium2 NeuronCores.

Strategy (data-parallel over batch):
  - hidden_states [8, 4096, 768] is sharded by batch element: core i gets
    x_i [4096, 768], pre-transposed on host to xT_i [768, 4096] so the
    hidden dim lives on SBUF partitions (6 chunks of 128) and no on-device
    transposes of the big tensor are needed.
  - All routing math (cosine/euclid sims, 4-layer similarity MLP, top-3
    selection + thresholding, weighted LoRA pooling, fusion weights) is
    computed on-device, replicated on every core (it is tiny).
  - The two LoRA branches (current-task + pooled) are fused into a single
    rank-16 LoRA:  y = x + x @ Ag.T @ Bg.T  with
       Ag = [c1*S*A_cur ; c2*S*pooled_a]  (rows),  Bg = [B_cur | pooled_b]
    Ag.T / Bg.T are assembled on-device with two small structured matmuls
    against the stacked LoRA banks (laG/lbG, (task,rank) packed on the
    128-partition axis).
  - Main loop per core: 8 tiles of 512 tokens; per tile
       uT[16,512]   = sum_c A_combT[c].T @ xT[c]     (fp32r matmuls)
       loraT[c]     = B_combT[c].T @ uT
       xT[c]       += loraT[c]                        (DVE, in-place)
    then DMA the tile back out.  Memory-bound: 12 MiB in + 12 MiB out/core.
"""

import numpy as np

B, S, H = 8, 4096, 768
N_TASKS, R = 16, 8
SCALING = 2.0
NCORES = 8
TPC = (B * S) // NCORES          # tokens per core = 4096
TT = 512                         # token tile
NTT = TPC // TT                  # 8 token tiles per core
NCH = H // 128                   # 6 hidden chunks
NR = N_TASKS * R                 # 128 = (task, rank) pairs

_PROGRAM = None


def _build_program():
    import concourse.bass as bass
    import concourse.tile as tile
    from concourse import bacc, mybir

    f32 = mybir.dt.float32
    f32r = mybir.dt.float32r
    AF = mybir.ActivationFunctionType
    OP = mybir.AluOpType
    AX = mybir.AxisListType

    nc = bacc.Bacc("TRN2", target_bir_lowering=False, debug=False)

    # ---- DRAM I/O ----
    def din(name, shape, dt=None):
        return nc.dram_tensor(name, shape, dt or f32, kind="ExternalInput").ap()

    xT = din("xT", [H, TPC], f32r)                 # per-core shard (transposed)
    teT = din("teT", [H, N_TASKS])
    te_row = din("te_row", [N_TASKS, H])
    curT = din("curT", [H, 1])
    cur_row = din("cur_row", [1, H])
    combT = din("combT", [2 * H, N_TASKS], f32r)
    W1T = din("W1T", [2 * H, 512], f32r)
    W2T = din("W2T", [512, 256], f32r)
    W3T = din("W3T", [256, 128])
    W4T = din("W4T", [128, 1])
    b1_row = din("b1_row", [1, 512])
    b2_row = din("b2_row", [1, 256])
    b3_row = din("b3_row", [1, 128])
    b4_row = din("b4_row", [1, 1])
    laG = din("laG", [NR, H])                # laG[n*8+r, h] = loras_a[n, r, h]
    lbG = din("lbG", [NR, H])                # lbG[n*8+r, h] = loras_b[n, h, r]
    M8 = din("M8", [NR, N_TASKS])            # M8[n*8+r, j] = (r == j % 8)
    E16 = din("E16", [N_TASKS, NR])          # E16[n, n*8+r] = 1
    ones128 = din("ones128", [1, NR])
    ones16 = din("ones16", [1, N_TASKS])
    ident16 = din("ident16", [16, 16])
    onehot_ext = din("onehot_ext", [NR, 1])  # (n == current_task_id), repeated 8x

    yT = nc.dram_tensor("yT", [H, TPC], f32, kind="ExternalOutput").ap()

    xT_r = xT.rearrange("(c p) t -> p c t", p=128)
    yT_r = yT.rearrange("(c p) t -> p c t", p=128)

    with tile.TileContext(nc) as tc:
        from contextlib import ExitStack
        with ExitStack() as ctx:
            const = ctx.enter_context(tc.tile_pool(name="const", bufs=1))
            pers = ctx.enter_context(tc.tile_pool(name="pers", bufs=1))
            pro = ExitStack()
            pp = pro.enter_context(tc.tile_pool(name="pp", bufs=3, space="PSUM"))
            bcp = pro.enter_context(tc.tile_pool(name="bcp", bufs=1, space="PSUM"))

            # ---- const loads ----
            def load(name, ap, shape, rearr=None, dt=None):
                t = const.tile(shape, dt or f32, name=name)
                nc.sync.dma_start(out=t, in_=ap if rearr is None else ap.rearrange(rearr, p=128))
                return t

            teT_sb = load("teT_sb", teT, [128, NCH, 16], "(c p) j -> p c j")
            curT_sb = load("curT_sb", curT, [128, NCH, 1], "(c p) j -> p c j")
            te_row_sb = load("te_row_sb", te_row, [16, H])
            cur_row_sb = load("cur_row_sb", cur_row, [1, H])
            combT_sb = load("combT_sb", combT, [128, 12, 16], "(c p) j -> p c j", dt=f32r)
            W1T_sb = load("W1T_sb", W1T, [128, 12, 512], "(c p) j -> p c j", dt=f32r)
            W2T_sb = load("W2T_sb", W2T, [128, 4, 256], "(c p) j -> p c j", dt=f32r)
            W3T_sb = load("W3T_sb", W3T, [128, 2, 128], "(c p) j -> p c j")
            W4T_sb = load("W4T_sb", W4T, [128, 1])
            b1_sb = load("b1_sb", b1_row, [1, 512])
            b2_sb = load("b2_sb", b2_row, [1, 256])
            b3_sb = load("b3_sb", b3_row, [1, 128])
            b4_sb = load("b4_sb", b4_row, [1, 1])
            laG_sb = load("laG_sb", laG, [128, H])
            lbG_sb = load("lbG_sb", lbG, [128, H])
            M8_sb = load("M8_sb", M8, [128, 16])
            E16_sb = load("E16_sb", E16, [16, 128])
            ones128_sb = load("ones128_sb", ones128, [1, 128])
            ones16_sb = load("ones16_sb", ones16, [1, 16])
            ident_sb = load("ident_sb", ident16, [16, 16])
            oh_sb = load("oh_sb", onehot_ext, [128, 1])

            # ================= routing prologue (replicated) =================
            # dots[n] = te[n] . cur   (and norms via ACT square+accum)
            dots_ps = pp.tile([16, 1], f32, tag="pp")
            for c in range(NCH):
                nc.tensor.matmul(dots_ps, lhsT=teT_sb[:, c, :], rhs=curT_sb[:, c, :],
                                 start=(c == 0), stop=(c == NCH - 1))
            dots = pers.tile([16, 1], f32)
            nc.scalar.copy(dots, dots_ps)

            scr_te = pers.tile([16, H], f32)
            te2 = pers.tile([16, 1], f32)
            nc.scalar.activation(scr_te, te_row_sb, AF.Square, accum_out=te2)
            scr_cur = pers.tile([1, H], f32)
            cur2 = pers.tile([1, 1], f32)
            nc.scalar.activation(scr_cur, cur_row_sb, AF.Square, accum_out=cur2)

            # broadcast cur2 to 16 partitions
            c2b_ps = pp.tile([16, 1], f32, tag="pp")
            nc.tensor.matmul(c2b_ps, lhsT=ones16_sb, rhs=cur2, start=True, stop=True)
            c2b = pers.tile([16, 1], f32)
            nc.scalar.copy(c2b, c2b_ps)

            emb_n = pers.tile([16, 1], f32)
            nc.scalar.sqrt(emb_n, te2)
            curn16 = pers.tile([16, 1], f32)
            nc.scalar.sqrt(curn16, c2b)

            den = pers.tile([16, 1], f32)
            nc.vector.tensor_mul(den, emb_n, curn16)
            nc.vector.tensor_scalar_max(den, den, 1e-8)
            rden = pers.tile([16, 1], f32)
            nc.vector.reciprocal(rden, den)
            cos = pers.tile([16, 1], f32)
            nc.vector.tensor_mul(cos, dots, rden)

            # euclid^2 = te2 - 2 dots + cur2  (clamped at 0)
            e2 = pers.tile([16, 1], f32)
            nc.vector.scalar_tensor_tensor(e2, in0=dots, scalar=-2.0, in1=te2,
                                           op0=OP.mult, op1=OP.add)
            nc.vector.tensor_add(e2, e2, c2b)
            nc.vector.tensor_scalar_max(e2, e2, 0.0)
            eu = pers.tile([16, 1], f32)
            nc.scalar.sqrt(eu, e2)
            eup1 = pers.tile([16, 1], f32)
            nc.scalar.add(eup1, eu, 1.0)
            es = pers.tile([16, 1], f32)
            nc.vector.reciprocal(es, eup1)

            # ---- similarity MLP ----
            h1_ps = pp.tile([16, 512], f32, tag="pp")
            for c in range(12):
                nc.tensor.matmul(h1_ps, lhsT=combT_sb[:, c, :],
                                 rhs=W1T_sb[:, c, :],
                                 start=(c == 0), stop=False)
            nc.tensor.matmul(h1_ps, lhsT=ones16_sb, rhs=b1_sb, start=False, stop=True)
            h1 = pers.tile([16, 512], f32)
            nc.scalar.activation(h1, h1_ps, AF.Relu)
            h1T = pers.tile([128, 4, 16], f32r)
            for c in range(4):
                tr_ps = pp.tile([128, 16], f32, tag="pp")
                nc.tensor.transpose(tr_ps, h1[:, c * 128:(c + 1) * 128], ident_sb)
                nc.scalar.copy(h1T[:, c, :], tr_ps)

            h2_ps = pp.tile([16, 256], f32, tag="pp")
            for c in range(4):
                nc.tensor.matmul(h2_ps, lhsT=h1T[:, c, :],
                                 rhs=W2T_sb[:, c, :],
                                 start=(c == 0), stop=False)
            nc.tensor.matmul(h2_ps, lhsT=ones16_sb, rhs=b2_sb, start=False, stop=True)
            h2 = pers.tile([16, 256], f32)
            nc.scalar.activation(h2, h2_ps, AF.Relu)
            h2T = pers.tile([128, 2, 16], f32)
            for c in range(2):
                tr_ps = pp.tile([128, 16], f32, tag="pp")
                nc.tensor.transpose(tr_ps, h2[:, c * 128:(c + 1) * 128], ident_sb)
                nc.scalar.copy(h2T[:, c, :], tr_ps)

            h3_ps = pp.tile([16, 128], f32, tag="pp")
            for c in range(2):
                nc.tensor.matmul(h3_ps, lhsT=h2T[:, c, :], rhs=W3T_sb[:, c, :],
                                 start=(c == 0), stop=False)
            nc.tensor.matmul(h3_ps, lhsT=ones16_sb, rhs=b3_sb, start=False, stop=True)
            h3 = pers.tile([16, 128], f32)
            nc.scalar.activation(h3, h3_ps, AF.Relu)
            h3T = pers.tile([128, 16], f32)
            tr_ps = pp.tile([128, 16], f32, tag="pp")
            nc.tensor.transpose(tr_ps, h3, ident_sb)
            nc.scalar.copy(h3T, tr_ps)

            z4_ps = pp.tile([16, 1], f32, tag="pp")
            nc.tensor.matmul(z4_ps, lhsT=h3T, rhs=W4T_sb, start=True, stop=False)
            nc.tensor.matmul(z4_ps, lhsT=ones16_sb, rhs=b4_sb, start=False, stop=True)
            nn_sim = pers.tile([16, 1], f32)
            nc.scalar.activation(nn_sim, z4_ps, AF.Sigmoid)

            # ---- sims = 0.4 cos + 0.3 es + 0.3 nn ----
            sims16 = pers.tile([16, 1], f32)
            nc.vector.scalar_tensor_tensor(sims16, in0=cos, scalar=0.4 / 0.3, in1=es,
                                           op0=OP.mult, op1=OP.add)
            nc.vector.tensor_add(sims16, sims16, nn_sim)
            nc.vector.tensor_scalar_mul(sims16, sims16, 0.3)

            sr_ps = pp.tile([1, 16], f32, tag="pp")
            nc.tensor.transpose(sr_ps, sims16, ident_sb)
            sims_row = pers.tile([1, 16], f32)
            nc.scalar.copy(sims_row, sr_ps)

            # ---- top-3 threshold ----
            m1 = pers.tile([1, 1], f32)
            nc.vector.reduce_max(m1, sims_row, axis=AX.X)
            msk = pers.tile([1, 16], f32)
            nc.vector.tensor_scalar(msk, in0=sims_row, scalar1=m1, scalar2=None, op0=OP.is_ge)
            s2 = pers.tile([1, 16], f32)
            nc.vector.scalar_tensor_tensor(s2, in0=msk, scalar=-1e30, in1=sims_row,
                                           op0=OP.mult, op1=OP.add)
            m2 = pers.tile([1, 1], f32)
            nc.vector.reduce_max(m2, s2, axis=AX.X)
            msk2 = pers.tile([1, 16], f32)
            nc.vector.tensor_scalar(msk2, in0=s2, scalar1=m2, scalar2=None, op0=OP.is_ge)
            s3 = pers.tile([1, 16], f32)
            nc.vector.scalar_tensor_tensor(s3, in0=msk2, scalar=-1e30, in1=s2,
                                           op0=OP.mult, op1=OP.add)
            m3 = pers.tile([1, 1], f32)
            nc.vector.reduce_max(m3, s3, axis=AX.X)

            ge3 = pers.tile([1, 16], f32)
            nc.vector.tensor_scalar(ge3, in0=sims_row, scalar1=m3, scalar2=None, op0=OP.is_ge)
            pos = pers.tile([1, 16], f32)
            nc.vector.tensor_scalar(pos, in0=sims_row, scalar1=0.0, scalar2=None, op0=OP.is_gt)
            m12 = pers.tile([1, 16], f32)
            nc.vector.tensor_mul(m12, ge3, pos)
            w_row = pers.tile([1, 16], f32)
            total = pers.tile([1, 1], f32)
            nc.vector.scalar_tensor_tensor(w_row, in0=m12, scalar=1.0, in1=sims_row,
                                           op0=OP.mult, op1=OP.mult, accum_out=total)

            tpos = pers.tile([1, 1], f32)
            nc.vector.tensor_scalar(tpos, in0=total, scalar1=0.0, scalar2=None, op0=OP.is_gt)
            tm1 = pers.tile([1, 1], f32)
            nc.vector.tensor_scalar_add(tm1, total, -1.0)
            safe = pers.tile([1, 1], f32)
            nc.vector.scalar_tensor_tensor(safe, in0=tm1, scalar=tpos, in1=ones16_sb[:, 0:1],
                                           op0=OP.mult, op1=OP.add)
            rinv = pers.tile([1, 1], f32)
            nc.vector.reciprocal(rinv, safe)
            wn_row = pers.tile([1, 16], f32)
            nc.vector.tensor_scalar_mul(wn_row, w_row, rinv)

            # fusion coefficients: c2 = min(0.1*||cur||, 0.5) * (total>0); c1 = 1-c2
            curn = pers.tile([1, 1], f32)
            nc.scalar.sqrt(curn, cur2)
            fw = pers.tile([1, 1], f32)
            nc.vector.tensor_scalar(fw, in0=curn, scalar1=0.1, scalar2=0.5,
                                    op0=OP.mult, op1=OP.min)
            cc = pers.tile([1, 2], f32)   # [c2*S | c1*S]
            c2v = pers.tile([1, 1], f32)
            nc.vector.tensor_mul(c2v, fw, tpos)
            nc.vector.tensor_scalar_mul(cc[:, 0:1], c2v, SCALING)
            nc.vector.tensor_scalar(cc[:, 1:2], in0=cc[:, 0:1], scalar1=-1.0, scalar2=SCALING,
                                    op0=OP.mult, op1=OP.add)
            ccb_ps = pp.tile([128, 2], f32, tag="pp")
            nc.tensor.matmul(ccb_ps, lhsT=ones128_sb, rhs=cc, start=True, stop=True)
            cc_b = pers.tile([128, 2], f32)
            nc.scalar.copy(cc_b, ccb_ps)

            # wn onto 128 (task,rank) partitions
            wc_ps = pp.tile([16, 1], f32, tag="pp")
            nc.tensor.transpose(wc_ps, wn_row, ident_sb[:1, :1])
            wn_col = pers.tile([16, 1], f32)
            nc.scalar.copy(wn_col, wc_ps)
            we_ps = pp.tile([128, 1], f32, tag="pp")
            nc.tensor.matmul(we_ps, lhsT=E16_sb, rhs=wn_col, start=True, stop=True)
            wn_ext = pers.tile([128, 1], f32)
            nc.scalar.copy(wn_ext, we_ps)

            # G2B (unscaled selector) and G2 (A-side, coefficient-scaled)
            G2B = pers.tile([128, 16], f32)
            nc.vector.tensor_scalar_mul(G2B[:, 0:8], M8_sb[:, 0:8], oh_sb)
            nc.vector.tensor_scalar_mul(G2B[:, 8:16], M8_sb[:, 8:16], wn_ext)
            G2 = pers.tile([128, 16], f32)
            nc.vector.tensor_scalar_mul(G2[:, 0:8], G2B[:, 0:8], cc_b[:, 1:2])
            nc.vector.tensor_scalar_mul(G2[:, 8:16], G2B[:, 8:16], cc_b[:, 0:1])

            # A_combT [768(c,p), 16] and B_combT [16, 768]
            A_comb = pers.tile([128, NCH, 16], f32r)
            for c in range(NCH):
                ac_ps = pp.tile([128, 16], f32, tag="pp")
                nc.tensor.matmul(ac_ps, lhsT=laG_sb[:, c * 128:(c + 1) * 128], rhs=G2,
                                 start=True, stop=True)
                nc.scalar.copy(A_comb[:, c, :], ac_ps)
            bc_ps = bcp.tile([16, H], f32)
            nc.tensor.matmul(bc_ps[:, 0:512], lhsT=G2B, rhs=lbG_sb[:, 0:512],
                             start=True, stop=True)
            nc.tensor.matmul(bc_ps[:, 512:768], lhsT=G2B, rhs=lbG_sb[:, 512:768],
                             start=True, stop=True)
            B_comb = pers.tile([16, H], f32r)
            nc.scalar.copy(B_comb, bc_ps)

            pro.close()

            # ================= main loop =================
            with (
                tc.tile_pool(name="xp", bufs=4) as xp,
                tc.tile_pool(name="yp", bufs=3) as yp,
                tc.tile_pool(name="usb", bufs=2) as usb,
                tc.tile_pool(name="ups", bufs=2, space="PSUM") as ups,
                tc.tile_pool(name="lps", bufs=4, space="PSUM") as lps,
            ):
                for it in range(NTT):
                    t0 = it * TT
                    xt = xp.tile([128, NCH, TT], f32r)
                    nc.sync.dma_start(out=xt, in_=xT_r[:, :, t0:t0 + TT])
                    u_ps = ups.tile([16, TT], f32)
                    for c in range(NCH):
                        nc.tensor.matmul(u_ps, lhsT=A_comb[:, c, :],
                                         rhs=xt[:, c, :],
                                         start=(c == 0), stop=(c == NCH - 1))
                    u_sb = usb.tile([16, TT], f32r)
                    nc.scalar.copy(u_sb, u_ps)
                    yt = yp.tile([128, NCH, TT], f32)
                    for c in range(NCH):
                        l_ps = lps.tile([128, TT], f32, tag="lora")
                        nc.tensor.matmul(l_ps, lhsT=B_comb[:, c * 128:(c + 1) * 128],
                                         rhs=u_sb, start=True, stop=True)
                        nc.vector.tensor_add(yt[:, c, :], xt[:, c, :].bitcast(f32), l_ps)
                    nc.sync.dma_start(out=yT_r[:, :, t0:t0 + TT], in_=yt)

    nc.compile()
    return nc


def _get_program():
    global _PROGRAM
    if _PROGRAM is None:
        _PROGRAM = _build_program()
    return _PROGRAM


def _make_in_maps(inputs):
    hs = np.ascontiguousarray(np.asarray(inputs["hidden_states"], np.float32))
    cur = np.ascontiguousarray(np.asarray(inputs["task_embedding"], np.float32))
    la = np.ascontiguousarray(np.asarray(inputs["loras_a"], np.float32))
    lb = np.ascontiguousarray(np.asarray(inputs["loras_b"], np.float32))
    te = np.ascontiguousarray(np.asarray(inputs["task_embeds"], np.float32))
    W1 = np.asarray(inputs["W1"], np.float32)
    W2 = np.asarray(inputs["W2"], np.float32)
    W3 = np.asarray(inputs["W3"], np.float32)
    W4 = np.asarray(inputs["W4"], np.float32)
    b1 = np.asarray(inputs["b1"], np.float32)
    b2 = np.asarray(inputs["b2"], np.float32)
    b3 = np.asarray(inputs["b3"], np.float32)
    b4 = np.asarray(inputs["b4"], np.float32)
    tid = int(np.asarray(inputs["current_task_id"]))

    idx = np.arange(NR)
    n_idx, r_idx = idx // R, idx % R
    M8 = np.zeros((NR, N_TASKS), np.float32)
    for j in range(N_TASKS):
        M8[:, j] = (r_idx == (j % R)).astype(np.float32)
    E16 = np.zeros((N_TASKS, NR), np.float32)
    E16[n_idx, idx] = 1.0
    onehot_ext = (n_idx == tid).astype(np.float32).reshape(NR, 1)

    rep = {
        "teT": np.ascontiguousarray(te.T),
        "te_row": te,
        "curT": np.ascontiguousarray(cur.reshape(H, 1)),
        "cur_row": np.ascontiguousarray(cur.reshape(1, H)),
        "combT": np.ascontiguousarray(
            np.concatenate([np.repeat(cur[:, None], N_TASKS, axis=1), te.T], axis=0)),
        "W1T": np.ascontiguousarray(W1.T),
        "W2T": np.ascontiguousarray(W2.T),
        "W3T": np.ascontiguousarray(W3.T),
        "W4T": np.ascontiguousarray(W4.T),
        "b1_row": np.ascontiguousarray(b1.reshape(1, 512)),
        "b2_row": np.ascontiguousarray(b2.reshape(1, 256)),
        "b3_row": np.ascontiguousarray(b3.reshape(1, 128)),
        "b4_row": np.ascontiguousarray(b4.reshape(1, 1)),
        "laG": np.ascontiguousarray(la.reshape(NR, H)),
        "lbG": np.ascontiguousarray(lb.transpose(0, 2, 1).reshape(NR, H)),
        "M8": M8,
        "E16": E16,
        "ones128": np.ones((1, NR), np.float32),
        "ones16": np.ones((1, N_TASKS), np.float32),
        "ident16": np.eye(16, dtype=np.float32),
        "onehot_ext": onehot_ext,
    }

    x2 = hs.reshape(B * S, H)
    in_maps = []
    for i in range(NCORES):
        shard = np.ascontiguousarray(x2[i * TPC:(i + 1) * TPC].T)  # [H, TPC]
        in_maps.append({"xT": shard, **rep})
    return in_maps


def kernel(**inputs):
    from concourse.bass_utils import run_bass_kernel_spmd

    nc = _get_program()
    in_maps = _make_in_maps(inputs)
    res = run_bass_kernel_spmd(nc, in_maps, core_ids=list(range(NCORES)))
    out = np.empty((B * S, H), np.float32)
    for i, r in enumerate(res.results):
        out[i * TPC:(i + 1) * TPC] = r["yT"].T
    return out.reshape(B, S, H)
